# revision 16
# baseline (speedup 1.0000x reference)
"""NT-Xent loss kernel for Trainium2, 8-core SPMD.

Math: with p = cat(z_i, z_j) [8192, 64], pn = p / max(||p||, 1e-8),
sim = 2 * pn @ pn.T (TEMP=0.5), the reference's gather-based losses reduce to
  loss1 = mean_r( log(sum_{c != r} exp(sim[r,c])) - pos_r )
  loss2 = mean_r( log(exp(pos_r) + sum_{c != t_r} exp(probs[r,c])) - pos_r )
where pos_r = sim[r, (r+N) % 2N].  sim entries lie in [-2, 2], so the exp
never overflows and no max-shift pass is needed.  The huge neg_idx input is a
fixed structured mask (drop self + positive) and never needs to be read.

Sharding: row-parallel.  Each of the 8 cores gets 1024 rows of the sim matrix,
computes sum_c exp(2 * pn_shard @ pn.T) against the full all-rows pn (computed
redundantly on every core from the full p), plus its rows' pos/diag terms and
the probs part, and emits two partial sums.  Host adds the 8 partials.
"""

import numpy as np

import concourse.bass as bass
import concourse.bacc as bacc
import concourse.tile as tile
from concourse import mybir
from concourse.masks import make_identity
from concourse.bass_utils import run_bass_kernel_spmd

N = 4096
D = 64
M = 2 * N            # 8192 rows of sim
NCORES = 8
R = M // NCORES      # 1024 rows per core
NT = M // 128        # 64 row-tiles of the full p
NS = R // 128        # 8 row-tiles of a shard
NCLS = 10
INV_TEMP = 2.0       # 1 / 0.5
F32 = mybir.dt.float32
BF16 = mybir.dt.bfloat16

# bf16 matmul for the sim slab: 4x PE throughput, 2x moving-dim. pos/diag
# stay fp32 (computed on DVE), and per-row errors average out over 8192 rows.
import os
USE_BF16_MM = os.environ.get("NTX_BF16", "0") == "1"

AF = mybir.ActivationFunctionType
ALU = mybir.AluOpType


def _emit_rsqrt(nc, pool, n2, nchunk):
    """inv = 1/max(sqrt(n2), 1e-8) per element, via exp(-0.5*ln(n2)) + one
    Newton step (keeps every ACT call in the natural_log_exp table set)."""
    tln = pool.tile([128, nchunk], F32, tag="rs_tln")
    inv0 = pool.tile([128, nchunk], F32, tag="rs_inv0")
    t2 = pool.tile([128, nchunk], F32, tag="rs_t2")
    inv = pool.tile([128, nchunk], F32, tag="rs_inv")
    nc.scalar.activation(tln, n2, AF.Ln)
    nc.scalar.activation(inv0, tln, AF.Exp, scale=-0.5)
    # Newton for rsqrt: y' = y * (1.5 - 0.5 * n2 * y^2)
    nc.vector.tensor_mul(t2, inv0, inv0)
    nc.vector.tensor_mul(t2, t2, n2)
    nc.vector.tensor_scalar(t2, t2, -0.5, 1.5, ALU.mult, ALU.add)
    nc.vector.tensor_mul(inv, inv0, t2)
    nc.vector.tensor_scalar_min(inv, inv, 1e8)
    return inv


def _emit_normalize(nc, pool, raw, ntiles, tag):
    """raw: [128, ntiles, 64] -> pn (same shape), rows normalized.

    The row scale is applied per 64-wide chunk with tensor_scalar_mul and a
    per-partition scalar AP (free-dim-broadcast APs with step 0 silently
    corrupt on HW, and tensor_tensor_reduce crashes the device).
    """
    flat = raw.rearrange("p n d -> p (n d)")
    sq = pool.tile([128, ntiles * D], F32, tag=f"{tag}_sq")
    n2 = pool.tile([128, ntiles], F32, tag=f"{tag}_n2")
    nc.vector.tensor_mul(sq, flat, flat)
    nc.vector.tensor_reduce(
        n2, sq.rearrange("p (n d) -> p n d", d=D), axis=mybir.AxisListType.X,
        op=ALU.add)
    inv = _emit_rsqrt(nc, pool, n2, ntiles)
    pn = pool.tile([128, ntiles, D], F32, tag=f"{tag}_pn")
    for n in range(ntiles):
        nc.vector.tensor_scalar_mul(pn[:, n, :], raw[:, n, :],
                                    inv[:, n:n + 1])
    return pn


def build_program():
    nc = bacc.Bacc("TRN2", target_bir_lowering=False, debug=False,
                   num_devices=NCORES)

    p_d = nc.dram_tensor("p", [M, D], F32, kind="ExternalInput").ap()
    ps_d = nc.dram_tensor("ps", [R, D], F32, kind="ExternalInput").ap()
    pp_d = nc.dram_tensor("pp", [R, D], F32, kind="ExternalInput").ap()
    probs_d = nc.dram_tensor("probs", [R, NCLS], F32, kind="ExternalInput").ap()
    iota_d = nc.dram_tensor("iotah", [128, NCLS], F32,
                            kind="ExternalInput").ap()
    tgtr_d = nc.dram_tensor("tgtrep", [128, NS, NCLS], F32,
                            kind="ExternalInput").ap()
    out_d = nc.dram_tensor("out", [1, 2], F32, kind="ExternalOutput").ap()

    with tile.TileContext(nc) as tc:
        import contextlib
        with contextlib.ExitStack() as ctx:
            consts = ctx.enter_context(tc.tile_pool(name="consts", bufs=1))
            big = ctx.enter_context(tc.tile_pool(name="big", bufs=1))
            work = ctx.enter_context(tc.tile_pool(name="work", bufs=2))
            tp = ctx.enter_context(
                tc.tile_pool(name="tp", bufs=2, space="PSUM"))
            mm = ctx.enter_context(
                tc.tile_pool(name="mm", bufs=2, space="PSUM"))
            po = ctx.enter_context(
                tc.tile_pool(name="po", bufs=1, space="PSUM"))
            esc = ctx.enter_context(tc.tile_pool(name="esc", bufs=2))

            identity = consts.tile([128, 128], F32)
            make_identity(nc, identity)
            iota10 = consts.tile([128, NCLS], F32)
            nc.sync.dma_start(out=iota10, in_=iota_d)
            ones = consts.tile([128, 1], F32)
            nc.vector.memset(ones, 1.0)

            # ---- Stage A: load + normalize the full p ----
            rawp = big.tile([128, NT, D], F32)
            nc.sync.dma_start(
                out=rawp, in_=p_d.rearrange("(n p) d -> p n d", p=128))
            pn = _emit_normalize(nc, big, rawp, NT, "p")

            # ---- Stage C: shard + partner slices ----
            rawps = big.tile([128, NS, D], F32)
            nc.sync.dma_start(
                out=rawps, in_=ps_d.rearrange("(n p) d -> p n d", p=128))
            pns = _emit_normalize(nc, big, rawps, NS, "s")
            rawpp = big.tile([128, NS, D], F32)
            nc.sync.dma_start(
                out=rawpp, in_=pp_d.rearrange("(n p) d -> p n d", p=128))
            pnp = _emit_normalize(nc, big, rawpp, NS, "q")

            MMDT = BF16 if USE_BF16_MM else F32

            # shard rows transposed: psT[d, r_local]  [64, 1024]
            psT = big.tile([64, R], MMDT)
            for g in range(NS // 4):
                tpp = tp.tile([64, 512], F32, tag="tp")
                for q in range(4):
                    nn = 4 * g + q
                    nc.tensor.transpose(
                        tpp[:, q * 128:(q + 1) * 128], pns[:, nn, :], identity)
                nc.vector.tensor_copy(psT[:, g * 512:(g + 1) * 512], tpp)

            # pos_r and diag_r row-dots (raw, without the *2 temp scale)
            diag_raw = big.tile([128, NS], F32)
            pos_raw = big.tile([128, NS], F32)
            dq = work.tile([128, NS, D], F32, tag="rowdot", bufs=2)
            nc.vector.tensor_mul(dq, pns, pns)
            nc.vector.tensor_reduce(diag_raw, dq, axis=mybir.AxisListType.X,
                                    op=ALU.add)
            pq = work.tile([128, NS, D], F32, tag="rowdot", bufs=2)
            nc.vector.tensor_mul(pq, pns, pnp)
            nc.vector.tensor_reduce(pos_raw, pq, axis=mybir.AxisListType.X,
                                    op=ALU.add)

            # ---- Stage B: full p transposed: pnT[d, r]  [64, 8192] ----
            pnT = big.tile([64, M], MMDT)
            for g in range(NT // 4):
                tpp = tp.tile([64, 512], F32, tag="tp")
                for q in range(4):
                    nn = 4 * g + q
                    nc.tensor.transpose(
                        tpp[:, q * 128:(q + 1) * 128], pn[:, nn, :], identity)
                nc.vector.tensor_copy(pnT[:, g * 512:(g + 1) * 512], tpp)

            # ---- Stage D: main loop  sum_c exp(2 * pn_shard @ pn.T) ----
            JJ = 8            # col groups of 1024
            scols = big.tile([128, NS * JJ], F32)
            for n in range(NS):
                lhsT = psT[:, n * 128:(n + 1) * 128]
                for jj in range(JJ):
                    pst = mm.tile([128, 1024], F32, tag="mm")
                    c0 = jj * 1024
                    nc.tensor.matmul(pst[:, 0:512], lhsT,
                                     pnT[:, c0:c0 + 512],
                                     start=True, stop=True)
                    nc.tensor.matmul(pst[:, 512:1024], lhsT,
                                     pnT[:, c0 + 512:c0 + 1024],
                                     start=True, stop=True)
                    et = esc.tile([128, 1024], F32, tag="esc")
                    nc.scalar.activation(
                        et, pst, AF.Exp, scale=INV_TEMP,
                        accum_out=scols[:, n * JJ + jj:n * JJ + jj + 1])

            # ---- loss1 pieces ----
            stot = big.tile([128, NS], F32)
            nc.vector.tensor_reduce(
                stot, scols.rearrange("p (n j) -> p n j", j=JJ),
                axis=mybir.AxisListType.X, op=ALU.add)
            ediag = big.tile([128, NS], F32)
            nc.scalar.activation(ediag, diag_raw, AF.Exp, scale=INV_TEMP)
            s1 = big.tile([128, NS], F32)
            nc.vector.tensor_sub(s1, stot, ediag)
            lse1 = big.tile([128, NS], F32)
            nc.scalar.activation(lse1, s1, AF.Ln)
            pos2 = big.tile([128, NS], F32)
            nc.vector.tensor_scalar_mul(pos2, pos_raw, INV_TEMP)
            c1 = big.tile([128, NS], F32)
            nc.vector.tensor_sub(c1, lse1, pos2)
            v12 = big.tile([128, 2], F32)
            nc.vector.tensor_reduce(v12[:, 0:1], c1,
                                    axis=mybir.AxisListType.X, op=ALU.add)

            # ---- Stage E: loss2 ----
            probs_t = big.tile([128, NS, NCLS], F32)
            nc.sync.dma_start(
                out=probs_t, in_=probs_d.rearrange("(n p) c -> p n c", p=128))
            tgtr_t = big.tile([128, NS, NCLS], F32)
            nc.sync.dma_start(out=tgtr_t, in_=tgtr_d)
            eprobs = big.tile([128, NS, NCLS], F32)
            nc.scalar.activation(
                eprobs.rearrange("p n c -> p (n c)"),
                probs_t.rearrange("p n c -> p (n c)"), AF.Exp)
            sum10 = big.tile([128, NS], F32)
            nc.vector.tensor_reduce(sum10, eprobs, axis=mybir.AxisListType.X,
                                    op=ALU.add)
            own = big.tile([128, NS], F32)
            for n in range(NS):
                mask = work.tile([128, NCLS], F32, tag="mask")
                nc.vector.tensor_tensor(mask, iota10, tgtr_t[:, n, :],
                                        ALU.is_equal)
                nc.vector.tensor_mul(mask, mask, eprobs[:, n, :])
                nc.vector.tensor_reduce(own[:, n:n + 1], mask,
                                        axis=mybir.AxisListType.X, op=ALU.add)
            epos = big.tile([128, NS], F32)
            nc.scalar.activation(epos, pos_raw, AF.Exp, scale=INV_TEMP)
            s2 = big.tile([128, NS], F32)
            nc.vector.tensor_sub(s2, sum10, own)
            nc.vector.tensor_add(s2, s2, epos)
            lse2 = big.tile([128, NS], F32)
            nc.scalar.activation(lse2, s2, AF.Ln)
            c2 = big.tile([128, NS], F32)
            nc.vector.tensor_sub(c2, lse2, pos2)
            nc.vector.tensor_reduce(v12[:, 1:2], c2,
                                    axis=mybir.AxisListType.X, op=ALU.add)

            # ---- Stage F: partition-sum via ones-matmul, then DMA out ----
            pso = po.tile([1, 2], F32)
            nc.tensor.matmul(pso, ones, v12, start=True, stop=True)
            outsb = big.tile([1, 2], F32)
            nc.vector.tensor_copy(outsb, pso)
            nc.sync.dma_start(out=out_d, in_=outsb)

    nc.compile()
    return nc


_NC_CACHE = None


def _get_nc():
    global _NC_CACHE
    if _NC_CACHE is None:
        _NC_CACHE = build_program()
    return _NC_CACHE


def make_in_maps(z_i, z_j, probs, target):
    p = np.ascontiguousarray(
        np.concatenate([z_i, z_j], axis=0), dtype=np.float32)
    t2 = np.concatenate([target, target]).astype(np.float32)
    probs = np.asarray(probs, dtype=np.float32)
    iotah = np.broadcast_to(np.arange(NCLS, dtype=np.float32),
                            (128, NCLS)).copy()
    in_maps = []
    for k in range(NCORES):
        lo = k * R
        plo = (lo + N) % M
        # tgtrep[p, n, c] = t2[lo + n*128 + p] for all c
        tgt_k = t2[lo:lo + R].reshape(NS, 128).T          # [128, NS]
        tgtrep = np.ascontiguousarray(
            np.repeat(tgt_k[:, :, None], NCLS, axis=2), dtype=np.float32)
        in_maps.append({
            "p": p,
            "ps": np.ascontiguousarray(p[lo:lo + R]),
            "pp": np.ascontiguousarray(p[plo:plo + R]),
            "probs": np.ascontiguousarray(probs[lo:lo + R]),
            "iotah": iotah,
            "tgtrep": tgtrep,
        })
    return in_maps


def kernel(z_i, z_j, probs, target, neg_idx):
    # neg_idx is the fixed structured NT-Xent mask (all columns except self and
    # positive); its effect is computed analytically, so it is never read.
    del neg_idx
    nc = _get_nc()
    in_maps = make_in_maps(np.asarray(z_i), np.asarray(z_j),
                           np.asarray(probs), np.asarray(target))
    res = run_bass_kernel_spmd(nc, in_maps, list(range(NCORES)))
    parts = np.stack([res.results[k]["out"].reshape(2) for k in range(NCORES)])
    total = parts.sum(axis=0) / np.float32(M)
    l1 = np.float32(total[0])
    l2 = np.float32(total[1])
    return (np.asarray(l1), np.asarray(l2))
